# revision 10
# baseline (speedup 1.0000x reference)
"""Trainium2 Bass kernel for BinaryMaskEdgeSmoothing.

Reference computation (per image, SAME-padded 3x3 convs):
    e  = conv3x3(x, lap)
    em = sigmoid(|e| * 3)
    b  = conv3x3(x, gauss)
    smoothed = x*(1-em) + b*em
    out = (smoothed > 0.5).astype(f32)

Key insight: the mask is binary, so every conv output is determined by
the 3x3 neighborhood pattern.  box = conv(x, ones3x3) and g16 =
16*conv(x, gauss) are small integers, and exhaustively checking all 512
neighborhood patterns against the f32 reference shows the whole
pipeline collapses to ONE linear threshold:

    out = [ conv3x3(x, K) > 10.125 ],
    K   = 16*gauss - 0.25*lap + 3.5*delta
        = [[1.25, 2.25, 1.25],
           [2.25, 5.50, 2.25],
           [1.25, 2.25, 1.25]]

(z = g16 + box/4 + 5x/4 uniquely separates the reference's decision
boundary, including its f32 sigmoid saturation behavior at |e|>=6 and
all zero-padded borders.)  All K values and x in {0,1} are exact in
fp8e4, products are exact, and PSUM accumulates in f32, so the kernel
is bit-exact vs the reference.

Device decomposition (per NeuronCore, data-parallel over B*C=64 images,
8 images per core):

  * Row-tiles of 128 input rows (partition dim), stride 126; the
    vertical direction of the conv is a banded-matrix matmul
    (lhsT[p, m] = K[dy, dx] at p = m+dy-1); horizontal taps are free-dim
    shifts of the moving operand.  Band truncation at partitions 0/127
    implements the zero padding at image top/bottom edges.
  * x is loaded as fp8 (host-cast, exact for a 0/1 mask) into a
    width-padded SBUF tile with zeroed guard columns, so horizontal
    shifts never need edge-special matmuls.
  * fp8 DoubleRow perf mode contracts 2 k-tiles per pass at 0.5
    cycles/output-row: the (dx=0, dx=2) taps pair into one matmul
    (their K columns are equal), and (dx=1, zero-band) forms the other.
    A 3x3 conv costs just 2 matmul instructions per 512-col PSUM chunk.
  * The threshold compare is a single DVE tensor_scalar is_gt reading
    PSUM f32 and writing the fp8 {0,1} output tile; output is stored as
    fp8 (8 MiB/core) and widened to f32 on the host.
  * Rows 1009..1023 of all 8 images batch into one final 128-partition
    tile with block-diagonal band matrices.

Engine budget per core: DMA ~16.2 MiB (~47 us at 358 GB/s, the
bottleneck), PE 260 DoubleRow matmuls (~28 us), DVE 130 compares
(~35 us).  Everything else idle.
"""

import numpy as np
import ml_dtypes

import concourse.bass as bass
import concourse.bacc as bacc
import concourse.mybir as mybir
import concourse.tile as tile
from concourse.bass_utils import run_bass_kernel_spmd

Op = mybir.AluOpType
Af = mybir.ActivationFunctionType
F32 = mybir.dt.float32
FP8 = mybir.dt.float8e4
NP_FP8 = mybir.dt.np(FP8)  # ml_dtypes.float8_e4m3

N_CORES = 8
B_PER_CORE = 8
H = 1024
W = 1024

WPAD = 1028  # padded SBUF row: x col j lives at c = 2 + j; c in {1,1026} zero
XOFF = 2
NMAIN = 8    # main row-tiles per image: in rows [126k, 126k+128), k=0..7
TAIL_S = 16  # tail block: in rows 1008..1023 of each image (out 1009..1023)
THRESH = 10.125


def _edge_kernel(lap_kernel, gauss_kernel):
    lap = np.asarray(lap_kernel, dtype=np.float64).reshape(3, 3)
    gau = np.asarray(gauss_kernel, dtype=np.float64).reshape(3, 3)
    K = 16.0 * gau - 0.25 * lap
    K[1, 1] += 3.5
    return K


def build_weights(lap_kernel, gauss_kernel):
    """Host-side: 8 banded [128,128] lhsT matrices -> [128, 8*128] fp8.

    Index i = variant*4 + pair*2 + t:
      variant 0: main band (p = m+dy-1), variant 1: tail block-diagonal
      pair 0: t=0 -> K col 0, t=1 -> K col 2   (dx=0 / dx=2 taps)
      pair 1: t=0 -> K col 1, t=1 -> zeros     (dx=1 tap)
    """
    K = _edge_kernel(lap_kernel, gauss_kernel)
    cols = {(0, 0): 0, (0, 1): 2, (1, 0): 1, (1, 1): None}
    mats = np.zeros((2, 2, 2, 128, 128), dtype=np.float64)
    for pair in range(2):
        for t in range(2):
            col = cols[(pair, t)]
            if col is None:
                continue
            m_main = mats[0, pair, t]
            for m in range(128):
                for dy in range(3):
                    p = m + dy - 1
                    if 0 <= p < 128:
                        m_main[p, m] = K[dy, col]
            m_tail = mats[1, pair, t]
            for blk in range(B_PER_CORE):
                base = blk * TAIL_S
                for ml in range(1, TAIL_S):
                    for dy in range(3):
                        p = ml + dy - 1
                        if 0 <= p < TAIL_S:
                            m_tail[base + p, base + ml] = K[dy, col]
    # stack as [p, i, m] so one DMA drops it straight into SBUF
    w = np.transpose(mats.reshape(8, 128, 128), (1, 0, 2)).reshape(128, 8 * 128)
    return np.ascontiguousarray(w.astype(np.float32)).astype(NP_FP8)


def build_nc(b_imgs=B_PER_CORE, h=H, w=W):
    # main units cover out rows 0..1008; tail covers 1009..1023
    assert 126 * (NMAIN - 1) + 126 + TAIL_S == h
    chunks = [(0, 512), (512, 1024)]
    PM = mybir.MatmulPerfMode.DoubleRow

    nc = bacc.Bacc()
    x_d = nc.declare_dram_parameter("x", [b_imgs, h, w], FP8, isOutput=False)
    w_d = nc.declare_dram_parameter("wts", [128, 8 * 128], FP8, isOutput=False)
    o_d = nc.declare_dram_parameter("out", [b_imgs, h, w], FP8, isOutput=True)

    with tile.TileContext(nc) as tc:
        with (
            tc.tile_pool(name="const", bufs=1) as cpool,
            tc.tile_pool(name="xin", bufs=3) as xpool,
            tc.tile_pool(name="oput", bufs=3) as opool,
            tc.tile_pool(name="psum", bufs=4, space="PSUM") as ppool,
        ):
            wsb = cpool.tile([128, 8, 128], FP8)
            nc.sync.dma_start(wsb[:], w_d[:])
            bias_t = cpool.tile([128, 1], F32)
            nc.gpsimd.memset(bias_t[:], -1024.0 * THRESH)

            def xrhs(xt, u, c0):
                """[128, 2, 512] moving AP: reads cols c0+2t+j of unit u."""
                ap = xt[:]
                pstride = ap.ap[0][0]
                return bass.AP(ap.tensor, u * WPAD + c0,
                               [[pstride, 128], [2, 2], [1, 512]])

            uctr = [0]

            def conv_unit(xt, u, variant, o_ap):
                """One [128, w] unit: 4 matmuls + 1 pointwise -> o_ap.

                The threshold compare round-robins over DVE / ACT / Pool
                so no single pointwise engine gates the PE.  On ACT it is
                sigmoid(1024*z - 1024*10.125): z is a multiple of 0.25, so
                the argument is always <= -128 or >= +128 and the result
                is exactly 0.0 / 1.0 after the fp8 cast.
                """
                ps = ppool.tile([128, w], F32, tag="ps")
                for (a, b) in chunks:
                    nc.tensor.matmul(
                        ps[:, a:b], wsb[:, variant * 4 + 0:variant * 4 + 2, :],
                        xrhs(xt, u, 1 + a), start=True, stop=False,
                        perf_mode=PM)
                    nc.tensor.matmul(
                        ps[:, a:b], wsb[:, variant * 4 + 2:variant * 4 + 4, :],
                        xrhs(xt, u, 2 + a), start=False, stop=True,
                        perf_mode=PM)
                # Pool/GpSimd cannot read PSUM, so only ACT and DVE share
                # the compares; 6:5 ratio equalizes their busy time.
                eng = "adadadadada"[uctr[0] % 11]
                uctr[0] += 1
                if eng == "a":
                    nc.scalar.activation(
                        o_ap[:], ps[:], Af.Sigmoid,
                        bias=bias_t[:], scale=1024.0)
                else:
                    nc.vector.tensor_scalar(
                        o_ap[:], ps[:], THRESH, None, Op.is_gt)

            SG = 4  # units per store group
            for b in range(b_imgs):
                xt = xpool.tile([128, NMAIN, WPAD], FP8, tag="xf")
                nc.sync.dma_start(
                    xt[:, :, XOFF:XOFF + w],
                    bass.AP(x_d, b * h * w, [[w, 128], [126 * w, NMAIN], [1, w]]))
                nc.gpsimd.memset(xt[:, :, 0:XOFF], 0.0)
                nc.gpsimd.memset(xt[:, :, XOFF + w:WPAD], 0.0)
                for k0 in range(0, NMAIN, SG):
                    o_grp = opool.tile([128, SG, w], FP8, tag="o")
                    for j in range(SG):
                        conv_unit(xt, k0 + j, 0, o_grp[:, j, :])
                    nc.scalar.dma_start(
                        bass.AP(o_d, (b * h + 126 * k0 + 1) * w,
                                [[w, 126], [126 * w, SG], [1, w]]),
                        o_grp[1:127, :, :])
                    if k0 == 0:
                        nc.scalar.dma_start(
                            o_d[b, 0:1, :], o_grp[0:1, 0, :])

            # tail: rows 1008..1023 of all images, block-diagonal bands
            xtt = xpool.tile([128, 1, WPAD], FP8, tag="xt")
            nc.sync.dma_start(
                xtt[:, 0, XOFF:XOFF + w], x_d[:, h - TAIL_S:h, :])
            nc.gpsimd.memset(xtt[:, :, 0:XOFF], 0.0)
            nc.gpsimd.memset(xtt[:, :, XOFF + w:WPAD], 0.0)
            o_t = opool.tile([128, w], FP8, tag="ot")
            conv_unit(xtt, 0, 1, o_t[:])
            for b in range(b_imgs):
                nc.scalar.dma_start(
                    o_d[b, h - TAIL_S + 1:h, :],
                    o_t[b * TAIL_S + 1:(b + 1) * TAIL_S, :])

    return nc


_NC_CACHE = {}


def _get_nc(key=(B_PER_CORE, H, W)):
    if key not in _NC_CACHE:
        nc = build_nc(*key)
        nc.finalize()
        _NC_CACHE[key] = nc
    return _NC_CACHE[key]


def make_in_maps(mask, lap_kernel, gauss_kernel):
    mask = np.asarray(mask)
    bb, cc, h, w = mask.shape
    assert (h, w) == (H, W) and bb * cc == N_CORES * B_PER_CORE
    x_all = np.ascontiguousarray(mask.reshape(N_CORES * B_PER_CORE, h, w))
    x_fp8 = x_all.astype(NP_FP8)
    wts = build_weights(lap_kernel, gauss_kernel)
    return [
        {"x": np.ascontiguousarray(x_fp8[c * B_PER_CORE:(c + 1) * B_PER_CORE]),
         "wts": wts}
        for c in range(N_CORES)
    ]


def kernel(mask, lap_kernel, gauss_kernel):
    mask = np.asarray(mask, dtype=np.float32)
    bb, cc, h, w = mask.shape
    in_maps = make_in_maps(mask, lap_kernel, gauss_kernel)
    nc = _get_nc()
    res = run_bass_kernel_spmd(nc, in_maps, list(range(N_CORES)))
    out = np.stack([res.results[c]["out"] for c in range(N_CORES)])
    return out.reshape(bb, cc, h, w).astype(np.float32)


# revision 14
# speedup vs baseline: 1.2151x; 1.2151x over previous
"""Trainium2 Bass kernel for BinaryMaskEdgeSmoothing.

Reference computation (per image, SAME-padded 3x3 convs):
    e  = conv3x3(x, lap)
    em = sigmoid(|e| * 3)
    b  = conv3x3(x, gauss)
    smoothed = x*(1-em) + b*em
    out = (smoothed > 0.5).astype(f32)

Key insight: the mask is binary, so every conv output is determined by
the 3x3 neighborhood pattern.  box = conv(x, ones3x3) and g16 =
16*conv(x, gauss) are small integers, and exhaustively checking all 512
neighborhood patterns against the f32 reference shows the whole
pipeline collapses to ONE linear threshold:

    out = [ conv3x3(x, K) > 10.125 ],
    K   = 16*gauss - 0.25*lap + 3.5*delta
        = [[1.25, 2.25, 1.25],
           [2.25, 5.50, 2.25],
           [1.25, 2.25, 1.25]]

(z = g16 + box/4 + 5x/4 uniquely separates the reference's decision
boundary, including its f32 sigmoid saturation behavior at |e|>=6 and
all zero-padded borders.)  All K values and x in {0,1} are exact in
fp8e4, products are exact, and PSUM accumulates in f32, so the kernel
is bit-exact vs the reference.

Device decomposition (per NeuronCore, data-parallel over B*C=64 images,
8 images per core):

  * Row-tiles of 128 input rows (partition dim), stride 126; the
    vertical direction of the conv is a banded-matrix matmul
    (lhsT[p, m] = K[dy, dx] at p = m+dy-1); horizontal taps are free-dim
    shifts of the moving operand.  Band truncation at partitions 0/127
    implements the zero padding at image top/bottom edges.
  * x is loaded as fp8 (host-cast, exact for a 0/1 mask) into a
    width-padded SBUF tile with zeroed guard columns, so horizontal
    shifts never need edge-special matmuls.
  * fp8 DoubleRow perf mode contracts 2 k-tiles per pass at 0.5
    cycles/output-row: the (dx=0, dx=2) taps pair into one matmul
    (their K columns are equal), and (dx=1, zero-band) forms the other.
    A 3x3 conv costs just 2 matmul instructions per 512-col PSUM chunk.
  * The threshold compare is a single DVE tensor_scalar is_gt reading
    PSUM f32 and writing the fp8 {0,1} output tile; output is stored as
    fp8 (8 MiB/core) and widened to f32 on the host.
  * Rows 1009..1023 of all 8 images batch into one final 128-partition
    tile with block-diagonal band matrices.

Engine budget per core: DMA ~16.2 MiB (~47 us at 358 GB/s, the
bottleneck), PE 260 DoubleRow matmuls (~28 us), DVE 130 compares
(~35 us).  Everything else idle.
"""

import numpy as np
import ml_dtypes

import concourse.bass as bass
import concourse.bacc as bacc
import concourse.mybir as mybir
import concourse.tile as tile
from concourse.bass_utils import run_bass_kernel_spmd

Op = mybir.AluOpType
Af = mybir.ActivationFunctionType
F32 = mybir.dt.float32
FP8 = mybir.dt.float8e4
NP_FP8 = mybir.dt.np(FP8)  # ml_dtypes.float8_e4m3

N_CORES = 8
B_PER_CORE = 8
H = 1024
W = 1024

WPAD = 1028  # padded SBUF row: x col j lives at c = 2 + j; c in {1,1026} zero
XOFF = 2
NMAIN = 8    # main row-tiles per image: in rows [126k, 126k+128), k=0..7
TAIL_S = 16  # tail block: in rows 1008..1023 of each image (out 1009..1023)
THRESH = 10.125


def _edge_kernel(lap_kernel, gauss_kernel):
    lap = np.asarray(lap_kernel, dtype=np.float64).reshape(3, 3)
    gau = np.asarray(gauss_kernel, dtype=np.float64).reshape(3, 3)
    K = 16.0 * gau - 0.25 * lap
    K[1, 1] += 3.5
    return K


def build_weights(lap_kernel, gauss_kernel):
    """Host-side: 8 banded [128,128] lhsT matrices -> [128, 8*128] fp8.

    Index i = variant*4 + pair*2 + t:
      variant 0: main band (p = m+dy-1), variant 1: tail block-diagonal
      pair 0: t=0 -> K col 0, t=1 -> K col 2   (dx=0 / dx=2 taps)
      pair 1: t=0 -> K col 1, t=1 -> zeros     (dx=1 tap)
    """
    K = _edge_kernel(lap_kernel, gauss_kernel)
    cols = {(0, 0): 0, (0, 1): 2, (1, 0): 1, (1, 1): None}
    mats = np.zeros((2, 2, 2, 128, 128), dtype=np.float64)
    for pair in range(2):
        for t in range(2):
            col = cols[(pair, t)]
            if col is None:
                continue
            m_main = mats[0, pair, t]
            for m in range(128):
                for dy in range(3):
                    p = m + dy - 1
                    if 0 <= p < 128:
                        m_main[p, m] = K[dy, col]
            # tail: input blocks of 16 partitions per image, outputs
            # remapped to contiguous partitions 1..120 (15 per image) so
            # the final store is a single partition-contiguous DMA
            m_tail = mats[1, pair, t]
            for blk in range(B_PER_CORE):
                for ml in range(1, TAIL_S):
                    for dy in range(3):
                        p = ml + dy - 1
                        if 0 <= p < TAIL_S:
                            m_tail[blk * TAIL_S + p,
                                   blk * (TAIL_S - 1) + ml] = K[dy, col]
    # stack as [p, i, m] so one DMA drops it straight into SBUF
    w = np.transpose(mats.reshape(8, 128, 128), (1, 0, 2)).reshape(128, 8 * 128)
    return np.ascontiguousarray(w.astype(np.float32)).astype(NP_FP8)


def build_nc(b_imgs=B_PER_CORE, h=H, w=W):
    # main units cover out rows 0..1008; tail covers 1009..1023
    assert 126 * (NMAIN - 1) + 126 + TAIL_S == h
    chunks = [(0, 512), (512, 1024)]
    PM = mybir.MatmulPerfMode.DoubleRow

    nc = bacc.Bacc()
    x_d = nc.declare_dram_parameter("x", [b_imgs, h, w], FP8, isOutput=False)
    w_d = nc.declare_dram_parameter("wts", [128, 8 * 128], FP8, isOutput=False)
    o_d = nc.declare_dram_parameter("out", [b_imgs, h, w], FP8, isOutput=True)

    with tile.TileContext(nc) as tc:
        with (
            tc.tile_pool(name="const", bufs=1) as cpool,
            tc.tile_pool(name="xin", bufs=4) as xpool,
            tc.tile_pool(name="oput", bufs=4) as opool,
            tc.tile_pool(name="psum", bufs=4, space="PSUM") as ppool,
        ):
            wsb = cpool.tile([128, 8, 128], FP8)
            nc.sync.dma_start(wsb[:], w_d[:])
            bias_t = cpool.tile([128, 1], F32)
            nc.gpsimd.memset(bias_t[:], -1024.0 * THRESH)

            def xrhs(xt, u, c0):
                """[128, 2, 512] moving AP: reads cols c0+2t+j of unit u."""
                ap = xt[:]
                pstride = ap.ap[0][0]
                return bass.AP(ap.tensor, u * WPAD + c0,
                               [[pstride, 128], [2, 2], [1, 512]])

            uctr = [0]

            def conv_unit(xt, u, variant, o_ap):
                """One [128, w] unit: 4 matmuls + 1 pointwise -> o_ap.

                The threshold compare round-robins over DVE / ACT / Pool
                so no single pointwise engine gates the PE.  On ACT it is
                sigmoid(1024*z - 1024*10.125): z is a multiple of 0.25, so
                the argument is always <= -128 or >= +128 and the result
                is exactly 0.0 / 1.0 after the fp8 cast.
                """
                ps = ppool.tile([128, w], F32, tag="ps")
                for (a, b) in chunks:
                    nc.tensor.matmul(
                        ps[:, a:b], wsb[:, variant * 4 + 0:variant * 4 + 2, :],
                        xrhs(xt, u, 1 + a), start=True, stop=False,
                        perf_mode=PM)
                    nc.tensor.matmul(
                        ps[:, a:b], wsb[:, variant * 4 + 2:variant * 4 + 4, :],
                        xrhs(xt, u, 2 + a), start=False, stop=True,
                        perf_mode=PM)
                # Pool/GpSimd cannot read PSUM, so only ACT and DVE share
                # the compares; 6:5 ratio equalizes their busy time.
                eng = "adadadadada"[uctr[0] % 11]
                uctr[0] += 1
                if eng == "a":
                    nc.scalar.activation(
                        o_ap[:], ps[:], Af.Sigmoid,
                        bias=bias_t[:], scale=1024.0)
                else:
                    nc.vector.tensor_scalar(
                        o_ap[:], ps[:], THRESH, None, Op.is_gt)

            SG = 2   # units per store group
            LH = 4   # units per load half (2 halves per image)

            # tail tile loads once, up front (prefetched while image 0
            # computes); block-diagonal bands consume it at the end.
            xtt = cpool.tile([128, 1, WPAD], FP8)

            for b in range(b_imgs):
                xt = xpool.tile([128, NMAIN, WPAD], FP8, tag="xf")
                for l0 in (0, LH):
                    nc.sync.dma_start(
                        xt[:, l0:l0 + LH, XOFF:XOFF + w],
                        bass.AP(x_d, (b * h + 126 * l0) * w,
                                [[w, 128], [126 * w, LH], [1, w]]))
                nc.gpsimd.memset(xt[:, :, 0:XOFF], 0.0)
                nc.gpsimd.memset(xt[:, :, XOFF + w:WPAD], 0.0)
                if b == 0:
                    nc.sync.dma_start(
                        xtt[:, 0, XOFF:XOFF + w], x_d[:, h - TAIL_S:h, :])
                    nc.gpsimd.memset(xtt[:, :, 0:XOFF], 0.0)
                    nc.gpsimd.memset(xtt[:, :, XOFF + w:WPAD], 0.0)
                for k0 in range(0, NMAIN, SG):
                    o_grp = opool.tile([128, SG, w], FP8, tag="o")
                    for j in range(SG):
                        conv_unit(xt, k0 + j, 0, o_grp[:, j, :])
                    nc.scalar.dma_start(
                        bass.AP(o_d, (b * h + 126 * k0 + 1) * w,
                                [[w, 126], [126 * w, SG], [1, w]]),
                        o_grp[1:127, :, :])
                    if k0 == 0:
                        nc.scalar.dma_start(
                            o_d[b, 0:1, :], o_grp[0:1, 0, :])

            # tail: rows 1009..1023 of all images land in partitions
            # 1..120 of o_t -> one contiguous batched store
            o_t = opool.tile([128, w], FP8, tag="ot")
            conv_unit(xtt, 0, 1, o_t[:])
            nc.scalar.dma_start(
                bass.AP(o_d, (h - TAIL_S + 1) * w,
                        [[h * w, b_imgs], [w, TAIL_S - 1], [1, w]]),
                o_t[1:1 + b_imgs * (TAIL_S - 1), :])

    return nc


_NC_CACHE = {}


def _get_nc(key=(B_PER_CORE, H, W)):
    if key not in _NC_CACHE:
        nc = build_nc(*key)
        nc.finalize()
        _NC_CACHE[key] = nc
    return _NC_CACHE[key]


def make_in_maps(mask, lap_kernel, gauss_kernel):
    mask = np.asarray(mask)
    bb, cc, h, w = mask.shape
    assert (h, w) == (H, W) and bb * cc == N_CORES * B_PER_CORE
    x_all = np.ascontiguousarray(mask.reshape(N_CORES * B_PER_CORE, h, w))
    x_fp8 = x_all.astype(NP_FP8)
    wts = build_weights(lap_kernel, gauss_kernel)
    return [
        {"x": np.ascontiguousarray(x_fp8[c * B_PER_CORE:(c + 1) * B_PER_CORE]),
         "wts": wts}
        for c in range(N_CORES)
    ]


def kernel(mask, lap_kernel, gauss_kernel):
    mask = np.asarray(mask, dtype=np.float32)
    bb, cc, h, w = mask.shape
    in_maps = make_in_maps(mask, lap_kernel, gauss_kernel)
    nc = _get_nc()
    res = run_bass_kernel_spmd(nc, in_maps, list(range(N_CORES)))
    out = np.stack([res.results[c]["out"] for c in range(N_CORES)])
    return out.reshape(bb, cc, h, w).astype(np.float32)
